# revision 1
# baseline (speedup 1.0000x reference)
"""Trainium2 Bass kernel for nn_GCNLayer (gnn_message_passing).

Strategy: pure data parallelism over the 128 graphs (16 graphs per core).
The spmm (segment_sum of vals*x[cols]) is computed block-dense: edges are
sorted by (row-window, col-window) on the host; the device builds 128x128
adjacency blocks S^T via iota/is_equal one-hots + PSUM matmuls, then
multiplies each block against SBUF-resident x tiles. All remaining phases
(alpha/beta/e3/new_e elementwise, attention pooling, final linear + tanh)
run on DVE/ACT/PE with matmul-based pooling and PE transposes.
"""
import sys
sys.path.insert(0, '/opt/trn_rl_repo')
import numpy as np
import ml_dtypes

NODES = 661
B_ALL = 128
GPC = 16                 # graphs per core
NCORE = 8
N = GPC * NODES          # 10576 nodes per core
NPAD = 10624             # 83 * 128
NWIN = NPAD // 128
F = 64
FX = 128                 # [e|f] fused width

bf16 = ml_dtypes.bfloat16


def _pack_core(rows, cols, vals, struct):
    """Pack one core's edges (local ids) into the common structure.
    struct: list over windows of list of (u, nch) blocks.
    Returns rl, cl, vl [128, nch_total] f32."""
    w = rows // 128
    u = cols // 128
    order = np.lexsort((u, w))
    r, c, v, wo, uo = rows[order], cols[order], vals[order], w[order], u[order]
    nch_total = sum(nch for win in struct for (_, nch) in win)
    rl = np.full((nch_total, 128), 128.0, np.float32)
    cl = np.zeros((nch_total, 128), np.float32)
    vl = np.zeros((nch_total, 128), np.float32)
    # index edges by (w,u)
    pos = 0
    ch = 0
    nE = len(r)
    for wi, win in enumerate(struct):
        for (uu, nch) in win:
            # edges for this (w,u)
            p0 = pos
            while pos < nE and wo[pos] == wi and uo[pos] == uu:
                pos += 1
            ne = pos - p0
            assert ne <= nch * 128, f"block overflow w={wi} u={uu}"
            fl_r = (r[p0:pos] - wi * 128).astype(np.float32)
            fl_c = (c[p0:pos] - uu * 128).astype(np.float32)
            fl_v = v[p0:pos].astype(np.float32)
            buf_r = np.full(nch * 128, 128.0, np.float32)
            buf_c = np.zeros(nch * 128, np.float32)
            buf_v = np.zeros(nch * 128, np.float32)
            buf_r[:ne] = fl_r; buf_c[:ne] = fl_c; buf_v[:ne] = fl_v
            rl[ch:ch + nch] = buf_r.reshape(nch, 128)
            cl[ch:ch + nch] = buf_c.reshape(nch, 128)
            vl[ch:ch + nch] = buf_v.reshape(nch, 128)
            ch += nch
        # skip any leftover edges of window wi not matching (shouldn't happen)
        while pos < nE and wo[pos] == wi:
            raise AssertionError("edge outside common structure")
    assert pos == nE
    return rl.T.copy(), cl.T.copy(), vl.T.copy()


def _common_struct(per_core_edges):
    """per_core_edges: list over cores of (rows, cols) local.
    Returns struct: list over windows of list of (u, nch)."""
    cnt = {}
    for rows, cols in per_core_edges:
        w = rows // 128
        u = cols // 128
        key = w.astype(np.int64) * 1024 + u
        uniq, c = np.unique(key, return_counts=True)
        for k, n in zip(uniq, c):
            k = int(k)
            cnt[k] = max(cnt.get(k, 0), int(n))
    struct = [[] for _ in range(NWIN)]
    for k in sorted(cnt):
        wi, uu = k // 1024, k % 1024
        nch = (cnt[k] + 127) // 128
        struct[wi].append((uu, nch))
    for wi in range(NWIN):
        if not struct[wi]:
            struct[wi].append((min(wi, NWIN - 1), 1))
    return struct


def _build_program(structs):
    """Build the Bass program. structs: dict set_name -> struct."""
    from concourse import bass, bacc, mybir, tile
    BF = mybir.dt.bfloat16
    F32 = mybir.dt.float32
    nc = bacc.Bacc("TRN2", target_bir_lowering=False, debug=False)

    sets = ["G", "B", "1", "2"]
    nch_tot = {s: sum(n for win in structs[s] for (_, n) in win) for s in sets}

    xbf_d = nc.dram_tensor("xbf", [NPAD, FX], BF, kind="ExternalInput")
    xf_d = nc.dram_tensor("xf", [NPAD, FX], F32, kind="ExternalInput")
    scal_d = nc.dram_tensor("scal", [NPAD, 4], F32, kind="ExternalInput")
    ind_d = nc.dram_tensor("ind", [NPAD, GPC], BF, kind="ExternalInput")
    indT_d = nc.dram_tensor("indT", [128, NPAD], BF, kind="ExternalInput")
    w1_d = nc.dram_tensor("w1", [5, 128, F], BF, kind="ExternalInput")
    w2_d = nc.dram_tensor("w2", [5, 128, F], BF, kind="ExternalInput")
    b1_d = nc.dram_tensor("b1", [128, F], BF, kind="ExternalInput")
    b2_d = nc.dram_tensor("b2", [128, F], BF, kind="ExternalInput")
    wa_d = nc.dram_tensor("wa", [128, 2], F32, kind="ExternalInput")  # [wae|waf] on 64+64 parts? cols: 0=wae,1=waf, rows 0:64 feats
    ba_d = nc.dram_tensor("ba", [GPC, 2], F32, kind="ExternalInput")
    meta_d = {s: {k: nc.dram_tensor(f"{k}{s}", [128, nch_tot[s]], BF, kind="ExternalInput")
                  for k in ("rl", "cl", "vl")} for s in sets}
    gset_d = {s: nc.dram_tensor(f"g{s}", [NPAD, FX], F32 if s in ("G", "B") else BF)
              for s in sets}
    enew_d = nc.dram_tensor("enew", [N, F], F32, kind="ExternalOutput")
    fnew_d = nc.dram_tensor("fnew", [N, F], F32, kind="ExternalOutput")

    with tile.TileContext(nc) as tc:
        with tile.TileContext.tile_pool(tc, name="const", bufs=1) as cpool:
            iota_t = cpool.tile([128, 128], BF)
            nc.gpsimd.iota(iota_t[:], pattern=[[1, 128]], base=0, channel_multiplier=0,
                           allow_small_or_imprecise_dtypes=True)
            iotap_t = cpool.tile([128, 1], mybir.dt.int32)
            nc.gpsimd.iota(iotap_t[:], pattern=[[1, 1]], base=0, channel_multiplier=1)
            iotap_bf = cpool.tile([128, 1], BF)
            nc.vector.tensor_copy(iotap_bf[:], iotap_t[:])
            ident_t = cpool.tile([128, 128], BF)
            nc.vector.tensor_tensor(out=ident_t[:], in0=iotap_bf[:].to_broadcast([128, 128]),
                                    in1=iota_t[:], op=mybir.AluOpType.is_equal)
            ones_t = cpool.tile([128, 128], BF)
            nc.vector.memset(ones_t[:], 1.0)
            xsb = cpool.tile([128, NWIN, FX], BF)
            nc.sync.dma_start(xsb[:], xbf_d[:].rearrange("(u p) f -> p u f", p=128))
            ind_bf = cpool.tile([128, NWIN, GPC], BF)
            nc.sync.dma_start(ind_bf[:], ind_d[:].rearrange("(u p) g -> p u g", p=128))
            indT_t = cpool.tile([128, NPAD], BF)
            nc.sync.dma_start(indT_t[:], indT_d[:])
            c3sb = cpool.tile([128, NWIN, FX], BF)
            cnsb = cpool.tile([128, NWIN, FX], BF)
            w1_t = cpool.tile([128, 5, F], BF)
            w2_t = cpool.tile([128, 5, F], BF)
            nc.sync.dma_start(w1_t[:], w1_d[:].rearrange("c p f -> p c f"))
            nc.sync.dma_start(w2_t[:], w2_d[:].rearrange("c p f -> p c f"))
            b1_t = cpool.tile([128, F], BF)
            b2_t = cpool.tile([128, F], BF)
            nc.sync.dma_start(b1_t[:], b1_d[:])
            nc.sync.dma_start(b2_t[:], b2_d[:])
            wa_t = cpool.tile([128, 2], F32)
            nc.sync.dma_start(wa_t[:], wa_d[:])
            ba_t = cpool.tile([GPC, 2], F32)
            nc.sync.dma_start(ba_t[:], ba_d[:])

            # ---------- spmm phase (4 sets) ----------
            with (tc.tile_pool(name="meta", bufs=1) as mpool,
                  tc.tile_pool(name="oh", bufs=3) as ohpool,
                  tc.tile_pool(name="st", bufs=6) as stpool,
                  tc.tile_pool(name="o", bufs=4) as opool,
                  tc.tile_pool(name="ps", bufs=4, space="PSUM") as pspool,
                  tc.tile_pool(name="pso", bufs=2, space="PSUM") as psopool):
                for s in sets:
                    struct = structs[s]
                    OD = F32 if s in ("G", "B") else BF
                    rl_t = mpool.tile([128, nch_tot[s]], BF, tag=f"rl{s}")
                    cl_t = mpool.tile([128, nch_tot[s]], BF, tag=f"cl{s}")
                    vl_t = mpool.tile([128, nch_tot[s]], BF, tag=f"vl{s}")
                    nc.sync.dma_start(rl_t[:], meta_d[s]["rl"][:])
                    nc.sync.dma_start(cl_t[:], meta_d[s]["cl"][:])
                    nc.sync.dma_start(vl_t[:], meta_d[s]["vl"][:])
                    ch0 = 0
                    for w in range(NWIN):
                        blocks = struct[w]
                        nch = sum(n for (_, n) in blocks)
                        ohr = ohpool.tile([128, nch, 128], BF, tag="ohr")
                        mc = ohpool.tile([128, nch, 128], BF, tag="mc")
                        nc.vector.tensor_tensor(
                            out=ohr[:], in0=rl_t[:, ch0:ch0 + nch].unsqueeze(2).broadcast_to([128, nch, 128]),
                            in1=iota_t[:].unsqueeze(1).broadcast_to([128, nch, 128]),
                            op=mybir.AluOpType.is_equal)
                        nc.vector.tensor_tensor(
                            out=mc[:], in0=cl_t[:, ch0:ch0 + nch].unsqueeze(2).broadcast_to([128, nch, 128]),
                            in1=iota_t[:].unsqueeze(1).broadcast_to([128, nch, 128]),
                            op=mybir.AluOpType.is_equal)
                        nc.vector.tensor_tensor(
                            out=mc[:], in0=mc[:],
                            in1=vl_t[:, ch0:ch0 + nch].unsqueeze(2).broadcast_to([128, nch, 128]),
                            op=mybir.AluOpType.mult)
                        ps_out = psopool.tile([128, FX], F32, space="PSUM", tag="pso")
                        jj = 0
                        for ui, (u, bn) in enumerate(blocks):
                            ps_st = pspool.tile([128, 128], F32, space="PSUM", tag="pst")
                            for si in range(bn):
                                nc.tensor.matmul(ps_st[:], lhsT=mc[:, jj + si, :], rhs=ohr[:, jj + si, :],
                                                 start=(si == 0), stop=(si == bn - 1))
                            st_sb = stpool.tile([128, 128], BF, tag="st")
                            if ui % 2 == 0:
                                nc.vector.tensor_copy(st_sb[:], ps_st[:])
                            else:
                                nc.scalar.copy(st_sb[:], ps_st[:])
                            nc.tensor.matmul(ps_out[:], lhsT=st_sb[:], rhs=xsb[:, u, :],
                                             start=(ui == 0), stop=(ui == len(blocks) - 1))
                            jj += bn
                        o = opool.tile([128, FX], OD, tag=f"o{s}")
                        nc.vector.tensor_copy(o[:], ps_out[:])
                        nc.sync.dma_start(gset_d[s][w * 128:(w + 1) * 128, :], o[:])
                        ch0 += nch

            # ---------- phase 2: elementwise + pooling ----------
            TT = mybir.AluOpType
            with (tc.tile_pool(name="p2", bufs=3) as p2,
                  tc.tile_pool(name="p2o", bufs=4) as p2o,
                  tc.tile_pool(name="ppool", bufs=1, space="PSUM") as ppool):
                pp3 = ppool.tile([128, GPC], F32, space="PSUM", tag="pp3")
                ppn = ppool.tile([128, GPC], F32, space="PSUM", tag="ppn")
                pp1 = ppool.tile([128, GPC], F32, space="PSUM", tag="pp1")
                pp2 = ppool.tile([128, GPC], F32, space="PSUM", tag="pp2")
                for w in range(NWIN):
                    sl = slice(w * 128, (w + 1) * 128)
                    x_t = p2.tile([128, FX], F32, tag="x")
                    gG_t = p2.tile([128, FX], F32, tag="gG")
                    gB_t = p2.tile([128, FX], F32, tag="gB")
                    g1_t = p2.tile([128, FX], BF, tag="g1")
                    g2_t = p2.tile([128, FX], BF, tag="g2")
                    sc_t = p2.tile([128, 4], F32, tag="sc")
                    nc.sync.dma_start(x_t[:], xf_d[sl, :])
                    nc.sync.dma_start(gG_t[:], gset_d["G"][sl, :])
                    nc.sync.dma_start(gB_t[:], gset_d["B"][sl, :])
                    nc.sync.dma_start(g1_t[:], gset_d["1"][sl, :])
                    nc.sync.dma_start(g2_t[:], gset_d["2"][sl, :])
                    nc.sync.dma_start(sc_t[:], scal_d[sl, :])
                    Pd, Qd, Gd, Bd = (sc_t[:, i:i + 1] for i in range(4))
                    e_, f_ = x_t[:, :F], x_t[:, F:]
                    sq = p2.tile([128, FX], F32, tag="sq")
                    nc.scalar.square(sq[:], x_t[:])
                    v2 = p2.tile([128, F], F32, tag="v2")
                    nc.vector.tensor_tensor(out=v2[:], in0=sq[:, :F], in1=sq[:, F:], op=TT.add)
                    base = p2.tile([128, F], F32, tag="base")
                    nc.vector.tensor_scalar_add(base[:], v2[:], 0.1)
                    rbase = p2.tile([128, F], F32, tag="rbase")
                    nc.vector.reciprocal(rbase[:], base[:])
                    gb2 = p2.tile([128, 2], F32, tag="gb2")
                    nc.vector.tensor_tensor(out=gb2[:], in0=sc_t[:, 2:4], in1=sc_t[:, 2:4], op=TT.mult)
                    gb = p2.tile([128, 1], F32, tag="gb")
                    nc.vector.tensor_tensor(out=gb[:], in0=gb2[:, 0:1], in1=gb2[:, 1:2], op=TT.add)
                    rgb = p2.tile([128, 1], F32, tag="rgb")
                    nc.vector.reciprocal(rgb[:], gb[:])
                    # alpha / beta
                    t1 = p2.tile([128, F], F32, tag="t1")
                    nc.scalar.activation(t1[:], e_, mybir.ActivationFunctionType.Copy, scale=Pd)
                    t2 = p2.tile([128, F], F32, tag="t2")
                    nc.scalar.activation(t2[:], f_, mybir.ActivationFunctionType.Copy, scale=Qd)
                    nc.vector.tensor_tensor(out=t1[:], in0=t1[:], in1=t2[:], op=TT.add)
                    nc.vector.tensor_tensor(out=t1[:], in0=t1[:], in1=rbase[:], op=TT.mult)
                    s1 = p2.tile([128, F], F32, tag="s1")
                    nc.vector.tensor_tensor(out=s1[:], in0=gG_t[:, :F], in1=gB_t[:, F:], op=TT.add)
                    alpha = p2.tile([128, F], F32, tag="alpha")
                    nc.vector.tensor_tensor(out=alpha[:], in0=t1[:], in1=s1[:], op=TT.subtract)
                    t3 = p2.tile([128, F], F32, tag="t3")
                    nc.scalar.activation(t3[:], e_, mybir.ActivationFunctionType.Copy, scale=Qd)
                    t4 = p2.tile([128, F], F32, tag="t4")
                    nc.scalar.activation(t4[:], f_, mybir.ActivationFunctionType.Copy, scale=Pd)
                    nc.vector.tensor_tensor(out=t3[:], in0=t3[:], in1=t4[:], op=TT.subtract)
                    nc.vector.tensor_tensor(out=t3[:], in0=t3[:], in1=rbase[:], op=TT.mult)
                    s2 = p2.tile([128, F], F32, tag="s2")
                    nc.vector.tensor_tensor(out=s2[:], in0=gG_t[:, F:], in1=gB_t[:, :F], op=TT.add)
                    beta = p2.tile([128, F], F32, tag="beta")
                    nc.vector.tensor_tensor(out=beta[:], in0=t3[:], in1=s2[:], op=TT.add)
                    # e3 / f3
                    c3_t = c3sb[:, w, :]
                    u1 = p2.tile([128, F], F32, tag="u1")
                    nc.scalar.activation(u1[:], alpha[:], mybir.ActivationFunctionType.Copy, scale=Gd)
                    u2 = p2.tile([128, F], F32, tag="u2")
                    nc.scalar.activation(u2[:], beta[:], mybir.ActivationFunctionType.Copy, scale=Bd)
                    nc.vector.tensor_tensor(out=u1[:], in0=u1[:], in1=u2[:], op=TT.add)
                    nc.vector.tensor_tensor(out=c3_t[:, 0:F], in0=u1[:], in1=rgb[:].to_broadcast([128, F]), op=TT.mult)
                    u3 = p2.tile([128, F], F32, tag="u3")
                    nc.scalar.activation(u3[:], beta[:], mybir.ActivationFunctionType.Copy, scale=Gd)
                    u4 = p2.tile([128, F], F32, tag="u4")
                    nc.scalar.activation(u4[:], alpha[:], mybir.ActivationFunctionType.Copy, scale=Bd)
                    nc.vector.tensor_tensor(out=u3[:], in0=u3[:], in1=u4[:], op=TT.subtract)
                    nc.vector.tensor_tensor(out=c3_t[:, F:FX], in0=u3[:], in1=rgb[:].to_broadcast([128, F]), op=TT.mult)
                    # new_e / new_f
                    base1 = p2.tile([128, F], F32, tag="base1")
                    nc.vector.tensor_tensor(out=base1[:], in0=gG_t[:, :F], in1=gB_t[:, F:], op=TT.subtract)
                    base2 = s2  # f_G + e_B already computed
                    vg = p2.tile([128, F], F32, tag="vg")
                    nc.scalar.activation(vg[:], v2[:], mybir.ActivationFunctionType.Copy, scale=Gd)
                    P_ = p2.tile([128, F], F32, tag="P_")
                    nc.vector.tensor_tensor(out=P_[:], in0=Pd.to_broadcast([128, F]), in1=vg[:], op=TT.subtract)
                    vb = p2.tile([128, F], F32, tag="vb")
                    nc.scalar.activation(vb[:], v2[:], mybir.ActivationFunctionType.Copy, scale=Bd)
                    Q_ = p2.tile([128, F], F32, tag="Q_")
                    nc.vector.tensor_tensor(out=Q_[:], in0=Qd.to_broadcast([128, F]), in1=vb[:], op=TT.add)
                    cn_t = cnsb[:, w, :]
                    n1 = p2.tile([128, F], F32, tag="n1")
                    nc.vector.tensor_tensor(out=n1[:], in0=P_[:], in1=base1[:], op=TT.mult)
                    n2 = p2.tile([128, F], F32, tag="n2")
                    nc.vector.tensor_tensor(out=n2[:], in0=Q_[:], in1=base2[:], op=TT.mult)
                    nc.vector.tensor_tensor(out=n1[:], in0=n1[:], in1=n2[:], op=TT.add)
                    nc.vector.tensor_tensor(out=cn_t[:, 0:F], in0=n1[:], in1=rgb[:].to_broadcast([128, F]), op=TT.mult)
                    n3 = p2.tile([128, F], F32, tag="n3")
                    nc.vector.tensor_tensor(out=n3[:], in0=P_[:], in1=base2[:], op=TT.mult)
                    n4 = p2.tile([128, F], F32, tag="n4")
                    nc.vector.tensor_tensor(out=n4[:], in0=Q_[:], in1=base1[:], op=TT.mult)
                    nc.vector.tensor_tensor(out=n3[:], in0=n3[:], in1=n4[:], op=TT.subtract)
                    nc.vector.tensor_tensor(out=cn_t[:, F:FX], in0=n3[:], in1=rgb[:].to_broadcast([128, F]), op=TT.mult)
                    # pools (accumulate over all windows)
                    nc.tensor.matmul(pp3[:], lhsT=c3_t, rhs=ind_bf[:, w, :], start=(w == 0), stop=(w == NWIN - 1))
                    nc.tensor.matmul(ppn[:], lhsT=cn_t, rhs=ind_bf[:, w, :], start=(w == 0), stop=(w == NWIN - 1))
                    nc.tensor.matmul(pp1[:], lhsT=g1_t[:], rhs=ind_bf[:, w, :], start=(w == 0), stop=(w == NWIN - 1))
                    nc.tensor.matmul(pp2[:], lhsT=g2_t[:], rhs=ind_bf[:, w, :], start=(w == 0), stop=(w == NWIN - 1))

                # ---------- phase 3: attention scalars ----------
                pools_sb = p2o.tile([128, 4 * GPC], F32, tag="pools")
                nc.vector.tensor_copy(pools_sb[:, 0 * GPC:1 * GPC], pp3[:])
                nc.vector.tensor_copy(pools_sb[:, 1 * GPC:2 * GPC], ppn[:])
                nc.vector.tensor_copy(pools_sb[:, 2 * GPC:3 * GPC], pp1[:])
                nc.vector.tensor_copy(pools_sb[:, 3 * GPC:4 * GPC], pp2[:])
                a_sb = p2o.tile([GPC, 8], F32, tag="a_sb")
                with tc.tile_pool(name="psc", bufs=2, space="PSUM") as pscp:
                    for ci in range(4):   # cand order: 3, n, 1, 2
                        for half in range(2):  # 0=e, 1=f
                            psc = pscp.tile([GPC, 1], F32, space="PSUM", tag="psc")
                            nc.tensor.matmul(psc[:],
                                             lhsT=pools_sb[:, ci * GPC:(ci + 1) * GPC],
                                             rhs=wa_t[:, half:half + 1],
                                             start=True, stop=True)
                            # a col layout: [e3,nf? ] -> col = half*4 + ci  (e cands 0..3, f cands 4..7)
                            nc.scalar.activation(a_sb[:, half * 4 + ci:half * 4 + ci + 1], psc[:],
                                                 mybir.ActivationFunctionType.Sigmoid,
                                                 bias=ba_t[:, half:half + 1], scale=1.0 / NODES)
                asum = p2o.tile([GPC, 2], F32, tag="asum")
                nc.vector.tensor_reduce(asum[:, 0:1], a_sb[:, 0:4], axis=mybir.AxisListType.X, op=TT.add)
                nc.vector.tensor_reduce(asum[:, 1:2], a_sb[:, 4:8], axis=mybir.AxisListType.X, op=TT.add)
                nc.vector.tensor_scalar_add(asum[:], asum[:], 1e-4)
                rasum = p2o.tile([GPC, 2], F32, tag="rasum")
                nc.vector.reciprocal(rasum[:], asum[:])
                # s8: pair order [e3,f3, ne,nf, e1,f1, e2,f2]
                s8 = p2o.tile([128, 8], BF, tag="s8")
                nc.vector.memset(s8[:], 0.0)
                for ci in range(4):
                    for half in range(2):
                        nc.vector.tensor_tensor(out=s8[:GPC, ci * 2 + half:ci * 2 + half + 1],
                                                in0=a_sb[:, half * 4 + ci:half * 4 + ci + 1],
                                                in1=rasum[:, half:half + 1], op=TT.mult)

            # ---------- phase 4: scale, transpose, final linear ----------
            with (tc.tile_pool(name="p4", bufs=3) as p4,
                  tc.tile_pool(name="p4t", bufs=6) as p4t,
                  tc.tile_pool(name="ps4", bufs=2, space="PSUM") as ps4,
                  tc.tile_pool(name="ps4o", bufs=2, space="PSUM") as ps4o):
                for w in range(NWIN):
                    sl = slice(w * 128, (w + 1) * 128)
                    nrows = min(N - w * 128, 128)
                    ps_s = ps4.tile([128, 8], F32, space="PSUM", tag="ps_s")
                    nc.tensor.matmul(ps_s[:], lhsT=indT_t[:, sl], rhs=s8[:], start=True, stop=True)
                    s_t = p4.tile([128, 8], F32, tag="s_t")
                    nc.vector.tensor_copy(s_t[:], ps_s[:])
                    g1t = p4.tile([128, FX], BF, tag="g1t")
                    g2t = p4.tile([128, FX], BF, tag="g2t")
                    nc.sync.dma_start(g1t[:], gset_d["1"][sl, :])
                    nc.sync.dma_start(g2t[:], gset_d["2"][sl, :])
                    scaled = []
                    for ci, srcap in enumerate((c3sb[:, w, :], cnsb[:, w, :], g1t[:], g2t[:])):
                        sc = p4t.tile([128, 2, F], BF, tag=f"sc{ci}")
                        nc.vector.tensor_tensor(
                            out=sc[:], in0=srcap.rearrange("p (h f) -> p h f", h=2),
                            in1=s_t[:, ci * 2:ci * 2 + 2].unsqueeze(2).broadcast_to([128, 2, F]),
                            op=TT.mult)
                        scaled.append(sc)
                    # transposes: scaled cands + x
                    xw = p4.tile([128, FX], BF, tag="xw")
                    nc.vector.tensor_copy(xw[:], xsb[:, w, :])
                    trs = []
                    for ci, src in enumerate(scaled + [xw]):
                        pst = ps4.tile([128, 128], F32, space="PSUM", tag="pst4")
                        ap = src[:].rearrange("p h f -> p (h f)") if ci < 4 else src[:]
                        nc.tensor.matmul(pst[:], lhsT=ap, rhs=ident_t[:], start=True, stop=True)
                        tr = p4t.tile([128, 128], BF, tag=f"tr{ci}")
                        if ci % 2 == 0:
                            nc.scalar.copy(tr[:], pst[:])
                        else:
                            nc.vector.tensor_copy(tr[:], pst[:])
                        trs.append(tr)
                    psE = ps4o.tile([128, F], F32, space="PSUM", tag="psE")
                    psF = ps4o.tile([128, F], F32, space="PSUM", tag="psF")
                    for ci in range(5):
                        nc.tensor.matmul(psE[:], lhsT=trs[ci][:], rhs=w1_t[:, ci, :],
                                         start=(ci == 0), stop=False)
                        nc.tensor.matmul(psF[:], lhsT=trs[ci][:], rhs=w2_t[:, ci, :],
                                         start=(ci == 0), stop=False)
                    nc.tensor.matmul(psE[:], lhsT=ones_t[:], rhs=b1_t[:], start=False, stop=True)
                    nc.tensor.matmul(psF[:], lhsT=ones_t[:], rhs=b2_t[:], start=False, stop=True)
                    oE = p4.tile([128, F], F32, tag="oE")
                    oF = p4.tile([128, F], F32, tag="oF")
                    nc.scalar.activation(oE[:], psE[:], mybir.ActivationFunctionType.Tanh)
                    nc.scalar.activation(oF[:], psF[:], mybir.ActivationFunctionType.Tanh)
                    nc.sync.dma_start(enew_d[w * 128:w * 128 + nrows, :], oE[:nrows, :])
                    nc.sync.dma_start(fnew_d[w * 128:w * 128 + nrows, :], oF[:nrows, :])
    nc.finalize()
    return nc


def kernel(e, f, rowsG, colsG, valsG, rowsB, colsB, valsB,
           rows1, cols1, vals1, rows2, cols2, vals2,
           G_diag, B_diag, Pd, Qd,
           W_v1, b_v1, W_v2, b_v2, w_ae, b_ae, w_af, b_af):
    from concourse.bass_utils import run_bass_kernel_spmd
    e = np.asarray(e); f = np.asarray(f)
    n_all = e.shape[0]
    sets_raw = {"G": (rowsG, colsG, valsG), "B": (rowsB, colsB, valsB),
                "1": (rows1, cols1, vals1), "2": (rows2, cols2, vals2)}
    # shard edges by owning core (graph id from row)
    per_core = {s: [] for s in sets_raw}
    for s, (rr, cc, vv) in sets_raw.items():
        rr = np.asarray(rr).astype(np.int64); cc = np.asarray(cc).astype(np.int64)
        vv = np.asarray(vv).astype(np.float32)
        core = rr // N
        for c in range(NCORE):
            m = core == c
            per_core[s].append((rr[m] - c * N, cc[m] - c * N, vv[m]))
    structs = {s: _common_struct([(pc[0], pc[1]) for pc in per_core[s]]) for s in sets_raw}
    nc = _build_program(structs)

    # indicator matrices
    ind = np.zeros((NPAD, GPC), np.float32)
    for g in range(GPC):
        ind[g * NODES:(g + 1) * NODES, g] = 1.0
    indT = np.zeros((128, NPAD), np.float32)
    indT[:GPC] = ind.T
    # weights: W [64, 320] -> 5 chunks W^T [64,64], replicated on 128 partitions
    def wchunks(W):
        out = np.zeros((5, 128, F), np.float32)
        for cidx in range(5):
            blk = W[:, cidx * F:(cidx + 1) * F].T  # [64 in, 64 out]
            out[cidx, :F] = blk
        return out
    w1 = wchunks(np.asarray(W_v1))
    w2raw = wchunks(np.asarray(W_v2))
    w2 = np.zeros_like(w2raw)
    w2[:, F:, :] = w2raw[:, :F, :]   # f-half rows carry W2, e-half rows zero
    w1 = w1.astype(bf16); w2 = w2.astype(bf16)
    wa = np.zeros((128, 2), np.float32)
    wa[:F, 0] = np.asarray(w_ae).reshape(-1)
    wa[F:, 1] = np.asarray(w_af).reshape(-1)
    ba = np.zeros((GPC, 2), np.float32)
    ba[:, 0] = float(np.asarray(b_ae).reshape(-1)[0])
    ba[:, 1] = float(np.asarray(b_af).reshape(-1)[0])

    in_maps = []
    for c in range(NCORE):
        sl = slice(c * N, (c + 1) * N)
        xf = np.zeros((NPAD, FX), np.float32)
        xf[:N, :F] = np.asarray(e)[sl]
        xf[:N, F:] = np.asarray(f)[sl]
        scal = np.zeros((NPAD, 4), np.float32)
        scal[:, 2:4] = 1.0
        scal[:N, 0] = np.asarray(Pd)[sl, 0]
        scal[:N, 1] = np.asarray(Qd)[sl, 0]
        scal[:N, 2] = np.asarray(G_diag)[sl, 0]
        scal[:N, 3] = np.asarray(B_diag)[sl, 0]
        im = {"xbf": xf.astype(bf16), "xf": xf, "scal": scal,
              "ind": ind.astype(bf16), "indT": indT.astype(bf16), "w1": w1, "w2": w2,
              "b1": np.tile(np.asarray(b_v1).reshape(1, F) / 128.0, (128, 1)).astype(bf16),
              "b2": np.tile(np.asarray(b_v2).reshape(1, F) / 128.0, (128, 1)).astype(bf16),
              "wa": wa, "ba": ba}
        for s in sets_raw:
            rl, cl, vl = _pack_core(*per_core[s][c], structs[s])
            im[f"rl{s}"] = rl.astype(bf16)
            im[f"cl{s}"] = cl.astype(bf16)
            im[f"vl{s}"] = vl.astype(bf16)
        in_maps.append(im)

    _BENCH_STATE['nc'] = nc
    _BENCH_STATE['in_maps'] = in_maps
    res = run_bass_kernel_spmd(nc, in_maps, list(range(NCORE)))
    e_new = np.concatenate([r["enew"] for r in res.results], axis=0)
    f_new = np.concatenate([r["fnew"] for r in res.results], axis=0)
    return e_new, f_new


_BENCH_STATE = {}


def bench(inputs, reps=12):
    if 'nc' not in _BENCH_STATE:
        kernel(**inputs)
    sys.path.insert(0, '/root/problem')
    from bench_util import bench_exec
    return bench_exec(_BENCH_STATE['nc'], _BENCH_STATE['in_maps'], NCORE, reps=reps)

